# revision 22
# baseline (speedup 1.0000x reference)
"""Trainium2 Bass kernel: multi-head attention with sequence-axis layernorm
and relative position bias, sharded 8-way over heads (2 heads/core).

Layout strategy (all per core):
  - LN over sequence axis done in [d_partition, n_free] layout (xT input);
    g is folded into Wq/Wkv on the host.
  - qT/kT produced transposed [inner_local=128, b*n] (head-dim on partitions)
    so sim is computed TRANSPOSED: simT[nj, ni] = kT.T @ qT (K=dh=64), with
    the two local heads row-tiled into the PE array concurrently (rows 0-63 /
    64-127 via base_partition auto tile_position).
  - the bias add is folded multiplicatively: the host precomputes
    exp(biasT) (bf16); on-chip attn_u = exp(sim) * expb via a DVE bf16
    multiply (2x mode) in the otherwise-idle attention window.
  - softmax without max-subtraction (scores ~ N(0,2); exp safe in f32);
    ScalarE exp reads PSUM [128, 1024] spans directly, writes bf16 attn_uT.
  - av matmul: lhsT = v_aug [nj, 65] (ones column -> row 64 = Z), rhs =
    attn_uT, accumulated over nj into [65, 512] PSUM chunks.
  - normalization by 1/Z is folded into the OUTPUT projection: per-head
    PSUM partials scaled by per-partition 1/Z columns (Z round-trips
    through DRAM to transpose rows->columns).
"""

import numpy as np
import ml_dtypes

import concourse.bass as bass
from concourse import bacc
import concourse.mybir as mybir
import concourse.tile as tile
from concourse.masks import make_identity
from concourse.bass_utils import run_bass_kernel_spmd

F32 = mybir.dt.float32
BF16 = mybir.dt.bfloat16
BF = ml_dtypes.bfloat16
AF = mybir.ActivationFunctionType
ALU = mybir.AluOpType

# full-size problem constants
B, N, DIM = 2, 2048, 1024
HEADS, DH = 16, 64
NCORES = 8
HL = HEADS // NCORES          # heads per core = 2
IL = HL * DH                  # local inner = 128
INNER = HEADS * DH            # 1024


def build(b_sz=B, n_sz=N, dim=DIM, eps=1e-5):
    """Build the per-core Bass graph (SPMD across 8 cores)."""
    nd = dim // 128               # d tiles
    nch = (b_sz * n_sz) // 512    # 512-col chunks of flattened b*n
    njb = n_sz // 128             # key tiles per batch
    nic = n_sz // 512             # query chunks per batch
    bn = b_sz * n_sz
    nsub = n_sz // 512            # bn_stats subgroups

    nc = bacc.Bacc(None, target_bir_lowering=False)
    xT = nc.declare_dram_parameter("xT", [b_sz, dim, n_sz], BF16, isOutput=False)
    wqT = nc.declare_dram_parameter("wqT", [dim, IL], BF16, isOutput=False)
    wkT = nc.declare_dram_parameter("wkT", [dim, IL], BF16, isOutput=False)
    wvT = nc.declare_dram_parameter("wvT", [dim, IL], BF16, isOutput=False)
    woT = nc.declare_dram_parameter("woT", [IL, dim], BF16, isOutput=False)
    biasT = nc.declare_dram_parameter("biasT", [HL, n_sz, n_sz], BF16, isOutput=False)  # holds exp(bias.T)
    out = nc.declare_dram_parameter("out", [bn, dim], F32, isOutput=True)
    zdram = nc.dram_tensor("zscratch", [b_sz, HL, n_sz], BF16)
    zrdram = nc.dram_tensor("zrscratch", [b_sz, HL, 1, n_sz], BF16)

    with tile.TileContext(nc) as tc:
        with (
            tc.tile_pool(name="consts", bufs=1) as consts,
            tc.tile_pool(name="persist", bufs=1) as persist,
        ):
            # ---- load weights; build identity ----
            wq_s, wk_s, wv_s = [], [], []
            for dt in range(nd):
                for lst, src, nm in ((wq_s, wqT, "wq"), (wk_s, wkT, "wk"), (wv_s, wvT, "wv")):
                    t = consts.tile([128, IL], BF16, tag=f"{nm}{dt}")
                    nc.sync.dma_start(out=t, in_=src[dt * 128:(dt + 1) * 128, :])
                    lst.append(t)
            ident = consts.tile([128, 128], BF16, tag="ident")
            make_identity(nc, ident)
            wo_h = []
            for h in range(HL):
                t = consts.tile([DH, dim], BF16, tag=f"wo{h}")
                nc.sync.dma_start(out=t, in_=woT[h * DH:(h + 1) * DH, :])
                wo_h.append(t)

            xn = {}
            qT = persist.tile([IL, bn], BF16, tag="qT")
            kT = persist.tile([IL, bn], BF16, tag="kT")
            va = {}
            avz = {}   # (b, h) -> [DH+1, n] bf16, rows 0..63 = av_u, row 64 = Z

            # ---- Phase 1: layernorm over sequence axis ----
            with (
                tc.tile_pool(name="xload", bufs=3) as xload,
                tc.tile_pool(name="lns", bufs=8) as lns,
            ):
                for b in range(b_sz):
                    for dt in range(nd):
                        xt = xload.tile([128, n_sz], BF16, tag="xt")
                        nc.sync.dma_start(out=xt, in_=xT[b, dt * 128:(dt + 1) * 128, :])
                        stats = lns.tile([128, nsub, 6], F32, tag="stats")
                        for s in range(nsub):
                            nc.vector.bn_stats(out=stats[:, s, :], in_=xt[:, s * 512:(s + 1) * 512])
                        mv = lns.tile([128, 2], F32, tag="mv")
                        nc.vector.bn_aggr(out=mv, in_=stats)
                        vmax = lns.tile([128, 1], F32, tag="vmax")
                        nc.vector.tensor_scalar_max(vmax, mv[:, 1:2], eps)
                        sq = lns.tile([128, 1], F32, tag="sq")
                        nc.scalar.activation(out=sq, in_=vmax, func=AF.Sqrt)
                        scl = lns.tile([128, 1], F32, tag="scl")
                        nc.vector.reciprocal(scl, sq)
                        shf = lns.tile([128, 1], F32, tag="shf")
                        nc.vector.tensor_mul(shf, mv[:, 0:1], scl)
                        nshf = lns.tile([128, 1], F32, tag="nshf")
                        nc.vector.tensor_scalar_mul(nshf, shf, -1.0)
                        xnt = persist.tile([128, n_sz], BF16, tag=f"xn_{b}_{dt}")
                        nc.scalar.activation(out=xnt, in_=xt, func=AF.Identity,
                                             bias=nshf, scale=scl)
                        xn[b, dt] = xnt

            # ---- Phase 2a: q/k projections (transposed layout) ----
            with tc.tile_pool(name="pproj", bufs=4, space="PSUM") as pproj:
                for ch in range(nch):
                    b = (ch * 512) // n_sz
                    col0 = (ch * 512) % n_sz
                    for (w_s, dst) in ((wq_s, qT), (wk_s, kT)):
                        ps = pproj.tile([IL, 512], F32, tag="ps")
                        for dt in range(nd):
                            nc.tensor.matmul(
                                ps, w_s[dt], xn[b, dt][:, col0:col0 + 512],
                                start=(dt == 0), stop=(dt == nd - 1),
                            )
                        nc.scalar.activation(out=dst[:, ch * 512:(ch + 1) * 512],
                                             in_=ps, func=AF.Copy)

            # ---- Phase 2b: v natural + ones column ----
            with tc.tile_pool(name="pv", bufs=4, space="PSUM") as pv:
                for b in range(b_sz):
                    for nj in range(njb):
                        psv = pv.tile([128, IL], F32, tag="psv")
                        for dt in range(nd):
                            nc.tensor.matmul(
                                psv, xn[b, dt][:, nj * 128:(nj + 1) * 128], wv_s[dt],
                                start=(dt == 0), stop=(dt == nd - 1),
                            )
                        for h in range(HL):
                            t = persist.tile([128, DH + 1], BF16, tag=f"va_{b}_{h}_{nj}")
                            nc.vector.tensor_copy(t[:, 0:DH], psv[:, h * DH:(h + 1) * DH])
                            nc.vector.memset(t[:, DH:DH + 1], 1.0)
                            va[b, h, nj] = t

            # ---- Phase 3+4: attention with interleaved output projection ----
            for b in range(b_sz):
                for h in range(HL):
                    avz[b, h] = persist.tile([DH + 1, n_sz], BF16, tag=f"avz_{b}_{h}",
                                             name=f"avz_{b}_{h}")
            with (
                tc.tile_pool(name="psim", bufs=1, space="PSUM") as psim,
                tc.tile_pool(name="pav", bufs=1, space="PSUM") as pavp,
                
                tc.tile_pool(name="attnp", bufs=4) as attnp,
                tc.tile_pool(name="biasp", bufs=6) as biasp,
                tc.tile_pool(name="ost", bufs=4) as ost,
                tc.tile_pool(name="zc", bufs=2) as zc,
            ):
                # both batches interleaved per round: 4 independent streams
                # (b x h) hide the sim->exp->mult->av latency; bias tile shared
                # across batches (same head/nj/ni)
                for ni in range(nic):
                    pavs = {}
                    for b in range(b_sz):
                        for h in range(HL):
                            pavs[b, h] = pavp.tile(
                                [DH + 1, 512], F32, tag=f"pav{b}_{h}",
                                name=f"pav_{b}_{ni}_{h}")
                    for nj in range(njb):
                        pst = {}
                        for b in range(b_sz):
                            for h in range(HL):
                                pst[b, h] = psim.tile([128, 512], F32,
                                                      tag=f"ps{b}_{h}",
                                                      name=f"ps_{b}_{ni}_{h}_{nj}")
                                nc.tensor.matmul(
                                    pst[b, h],
                                    kT[h * DH:(h + 1) * DH,
                                       b * n_sz + nj * 128:b * n_sz + (nj + 1) * 128],
                                    qT[h * DH:(h + 1) * DH,
                                       b * n_sz + ni * 512:b * n_sz + (ni + 1) * 512],
                                    start=True, stop=True,
                                )
                        aus = {}
                        for h in range(HL):
                            bt = biasp.tile([128, 512], BF16, tag="bt", name="bt")
                            nc.sync.dma_start(
                                out=bt,
                                in_=biasT[h, nj * 128:(nj + 1) * 128,
                                          ni * 512:(ni + 1) * 512],
                            )
                            for b in range(b_sz):
                                ae = attnp.tile([128, 512], BF16,
                                                tag=f"ae{b}_{h}", name="ae")
                                nc.scalar.activation(out=ae, in_=pst[b, h], func=AF.Exp)
                                au = attnp.tile([128, 512], BF16,
                                                tag=f"au{b}_{h}", name="au")
                                nc.vector.tensor_mul(au, ae, bt)
                                aus[b, h] = au
                        for b in range(b_sz):
                            for h in range(HL):
                                nc.tensor.matmul(
                                    pavs[b, h], va[b, h, nj], aus[b, h],
                                    start=(nj == 0), stop=(nj == njb - 1),
                                )
                    for b in range(b_sz):
                        for h in range(HL):
                            nc.vector.tensor_copy(
                                avz[b, h][:, ni * 512:(ni + 1) * 512], pavs[b, h])
                # ---- Z -> per-partition 1/Z columns (runs concurrent with out MMs) ----
                zrec = {}
                for b in range(b_sz):
                    for h in range(HL):
                        nc.sync.dma_start(out=zdram[b, h, :], in_=avz[b, h][DH:DH + 1, :])
                    zcol = zc.tile([128, HL, njb], BF16, tag=f"zcol{b}", name="zcol")
                    nc.sync.dma_start(
                        out=zcol, in_=zdram[b].rearrange("h (c p) -> p h c", p=128))
                    zr = zc.tile([128, HL, njb], F32, tag=f"zrb{b}", name="zrb")
                    nc.vector.reciprocal(zr, zcol)
                    zrec[b] = zr
            with (
                tc.tile_pool(name="pout", bufs=2, space="PSUM") as pout,
                tc.tile_pool(name="ost2", bufs=3) as ost2,
            ):
                for blk in range(bn // 128):
                    b = (blk * 128) // n_sz
                    r0 = (blk * 128) % n_sz
                    jb = r0 // 128
                    po = {}
                    for h in range(HL):
                        po[h] = pout.tile([128, dim], F32, tag=f"po{h}", name="po")
                        for c0 in range(0, dim, 512):
                            w = min(512, dim - c0)
                            nc.tensor.matmul(
                                po[h][:, c0:c0 + w],
                                avz[b, h][0:DH, r0:r0 + 128],
                                wo_h[h][:, c0:c0 + w],
                                start=True, stop=True,
                            )
                    os_ = ost2.tile([128, dim], F32, tag="os", name="os")
                    nc.vector.tensor_scalar_mul(os_, po[0], zrec[b][:, 0, jb:jb + 1])
                    nc.vector.scalar_tensor_tensor(
                        out=os_, in0=po[1], scalar=zrec[b][:, 1, jb:jb + 1],
                        in1=os_, op0=ALU.mult, op1=ALU.add,
                    )
                    nc.sync.dma_start(out=out[blk * 128:(blk + 1) * 128, :], in_=os_)
    nc.compile()
    return nc


_NC_CACHE = {}


def _get_nc(key, **kw):
    if key not in _NC_CACHE:
        _NC_CACHE[key] = build(**kw)
    return _NC_CACHE[key]


def make_in_maps(x, rel_pos_bias, g, Wq, Wkv, Wo):
    b_sz, n_sz, dim = x.shape
    inner = Wq.shape[0]
    x = np.asarray(x, np.float32)
    xTh = np.ascontiguousarray(x.transpose(0, 2, 1)).astype(BF)  # [B, DIM, N]
    gv = np.asarray(g, np.float32).reshape(1, dim)
    Wq = np.asarray(Wq, np.float32) * gv
    Wkv = np.asarray(Wkv, np.float32) * gv
    scale = DH ** -0.5
    in_maps = []
    for c in range(NCORES):
        rs, re = c * IL, (c + 1) * IL
        wq_c = np.ascontiguousarray((Wq[rs:re, :] * scale).T).astype(BF)
        wk_c = np.ascontiguousarray(Wkv[rs:re, :].T).astype(BF)
        wv_c = np.ascontiguousarray(Wkv[inner + rs:inner + re, :].T).astype(BF)
        wo_c = np.ascontiguousarray(np.asarray(Wo)[:, rs:re].T).astype(BF)
        bias_c = np.exp(np.ascontiguousarray(
            np.asarray(rel_pos_bias)[0, c * HL:(c + 1) * HL].transpose(0, 2, 1)
        )).astype(BF)
        in_maps.append({
            "xT": xTh, "wqT": wq_c, "wkT": wk_c, "wvT": wv_c,
            "woT": wo_c, "biasT": bias_c,
        })
    return in_maps


def kernel(x, rel_pos_bias, g, Wq, Wkv, Wo):
    b_sz, n_sz, dim = x.shape
    nc = _get_nc((b_sz, n_sz, dim), b_sz=b_sz, n_sz=n_sz, dim=dim)
    in_maps = make_in_maps(x, rel_pos_bias, g, Wq, Wkv, Wo)
    res = run_bass_kernel_spmd(nc, in_maps, core_ids=list(range(NCORES)))
    acc = np.zeros((b_sz * n_sz, dim), np.float32)
    for r in res.results:
        acc += np.asarray(r["out"], np.float32)
    return np.ascontiguousarray(acc.reshape(b_sz, n_sz, dim))
